# revision 6
# baseline (speedup 1.0000x reference)
"""Trainium2 Bass kernel for the AttentiveTransformer block:
    mask = sparsemax(BN(inputs @ W + b) * prior)

Contract: kernel(**inputs) takes FULL unsharded numpy inputs and returns the
FULL [65536, 512] float32 output. The batch axis is sharded over 8
NeuronCores (pure data parallelism, 8192 rows each); the small Dense/BN
params are replicated to every core. No cross-core communication is needed
(sparsemax is row-wise).

Host-side prep (cheap, O(B*D)): BatchNorm (inference) is folded into the
dense layer; inputs are pre-transposed to [D, B] bf16 so the contraction dim
lands on partitions with no on-device transpose. A single bf16 matmul is
used (z error ~5e-3 absmax, inside the 2e-2 budget).

Device algorithm per 128-row tile (rows on partitions, F=512 on free axis):
  1. PE: one bf16 matmul -> PSUM fp32 (z stays resident in PSUM).
  2. DVE: one max8 over the 512 columns -> top-8 candidates in SBUF.
  3. GpSimd (Pool): batched simplex-threshold math on the candidates,
     tau_hat = max_j (csum_j - 1)/(j+1) over j<ncand. Because tau from ANY
     value-subset of a row lower-bounds the true sparsemax tau, tau_hat <=
     tau* always; rows with support > ncand get tau_hat < tau*.
  4. ACT: out = Relu(z + (-tau_hat)) straight from PSUM, written bf16.
  5. Batched 4-tile slab DMAs in/out on the sync HWDGE ring.

Host post-pass: rows whose written mask sums to > 1 + eps are exactly those
where tau_hat < tau* (plus harmless rounding flags); for those (~7%) the
host runs an exact sparsemax on the row's nonzeros, which provably recovers
the true projection of the device z. Re-fixing an already-correct row is a
no-op, so over-flagging is safe. End-to-end absmax err ~7e-3 (gate 2e-2).

Input-dependent specialization (checked on host at call time):
  * folded BN bias is zero for this problem -> bias rank-1 matmul elided;
  * `prior` is all-ones (spec fill=ones) -> prior load/multiply skipped.
  Program variants exist for both non-default cases.
"""

import numpy as np

B, D, F = 65536, 128, 512
NCORES = 8
RPC = B // NCORES        # rows per core
NT = RPC // 128          # 128-row tiles per core (64)
TPS = 4                  # tiles per slab (= PSUM batch for threshold math)
NS = NT // TPS           # slabs per core (16)
NCAND = 8                # top-k candidates per row for tau_hat
BN_EPS = 1e-3
SUM_TOL = 2e-3           # host-fix flag threshold on row sums

_CACHE = {}


def _build_program(use_bias, use_prior):
    import concourse.bass as bass
    import concourse.bacc as bacc
    import concourse.mybir as mybir
    from concourse.tile import TileContext

    f32 = mybir.dt.float32
    bf16 = mybir.dt.bfloat16
    Alu = mybir.AluOpType
    Act = mybir.ActivationFunctionType

    nc = bacc.Bacc("TRN2", target_bir_lowering=False)
    xt_d = nc.dram_tensor("xt", [D, RPC], bf16, kind="ExternalInput")
    w_d = nc.dram_tensor("w", [D, F], bf16, kind="ExternalInput")
    kj_d = nc.dram_tensor("kj", [1, TPS * NCAND], f32, kind="ExternalInput")
    if use_bias:
        cv_d = nc.dram_tensor("cv", [2, F], bf16, kind="ExternalInput")
    if use_prior:
        pr_d = nc.dram_tensor("prior", [RPC, F], f32, kind="ExternalInput")
    out_d = nc.dram_tensor("out", [RPC, F], bf16, kind="ExternalOutput")

    with TileContext(nc) as tc:
        with (
            tc.tile_pool(name="consts", bufs=1) as consts,
            tc.tile_pool(name="xin", bufs=3) as xin_pool,
            tc.tile_pool(name="psum", bufs=8, space="PSUM") as psum_pool,
            tc.tile_pool(name="obuf", bufs=3) as o_pool,
            tc.tile_pool(name="smallw", bufs=2) as sw_pool,
            tc.tile_pool(name="smallt", bufs=2) as st_pool,
            tc.tile_pool(name="zbuf", bufs=2 * TPS + 2) as z_pool,
        ):
            w_sb = consts.tile([D, F], bf16)
            nc.sync.dma_start(out=w_sb, in_=w_d[:, :])
            if use_bias:
                cv_sb = consts.tile([2, F], bf16)
                nc.sync.dma_start(out=cv_sb, in_=cv_d[:, :])
                ones_sb = consts.tile([2, D], bf16)
                nc.vector.memset(ones_sb, 1.0)
            # 1/(j+1) rule coefficients (tiled per slab-tile) replicated
            # to all 128 partitions
            kj_sb = consts.tile([128, TPS * NCAND], f32)
            kj_bcast = bass.AP(
                tensor=kj_d, offset=0, ap=[[0, 128]] + kj_d[0:1, :].ap[1:]
            )
            nc.sync.dma_start(out=kj_sb, in_=kj_bcast)
            # all candidates + all negated thresholds live in one tile each
            cand_sb = consts.tile([128, NT * NCAND], f32)
            ntau_sb = consts.tile([128, NT], f32)

            prev = None
            for s in range(NS):
                xin = xin_pool.tile([D, TPS * 128], bf16)
                nc.sync.dma_start(
                    out=xin, in_=xt_d[:, s * TPS * 128:(s + 1) * TPS * 128]
                )
                z_list = []
                for j in range(TPS):
                    t = s * TPS + j
                    xps = psum_pool.tile([128, F], f32)
                    nc.tensor.matmul(
                        xps, lhsT=xin[:, j * 128:(j + 1) * 128], rhs=w_sb[:, :],
                        start=True, stop=not use_bias,
                    )
                    if use_bias:
                        nc.tensor.matmul(
                            xps, lhsT=ones_sb[:, :], rhs=cv_sb[:, :],
                            start=False, stop=True,
                        )
                    if use_prior:
                        pr_t = xin_pool.tile([128, F], f32, tag="pr")
                        nc.sync.dma_start(
                            out=pr_t, in_=pr_d[t * 128:(t + 1) * 128, :]
                        )
                        z = z_pool.tile([128, F], f32)
                        nc.vector.tensor_tensor(
                            out=z, in0=xps, in1=pr_t, op=Alu.mult
                        )
                    else:
                        z = xps
                    nc.vector.max(
                        out=cand_sb[:, t * NCAND:(t + 1) * NCAND], in_=z
                    )
                    z_list.append(z)

                # ---- batched threshold math on GpSimd (SBUF-only engine) ----
                # Pool's legal op set excludes scan/scalar_tensor_tensor, so
                # the per-8-segment cumsum is a Hillis-Steele ladder of
                # shifted tensor_tensor adds on 3D views.
                seg = lambda ap: ap.rearrange("p (t s) -> p t s", s=NCAND)
                cnd3 = seg(cand_sb[:, s * TPS * NCAND:(s + 1) * TPS * NCAND])
                c1 = sw_pool.tile([128, TPS * NCAND], f32, tag="c1")
                c13 = seg(c1)
                nc.gpsimd.tensor_copy(out=c13[:, :, 0:1], in_=cnd3[:, :, 0:1])
                nc.gpsimd.tensor_tensor(
                    out=c13[:, :, 1:8], in0=cnd3[:, :, 1:8],
                    in1=cnd3[:, :, 0:7], op=Alu.add,
                )
                c2 = sw_pool.tile([128, TPS * NCAND], f32, tag="c2")
                c23 = seg(c2)
                nc.gpsimd.tensor_copy(out=c23[:, :, 0:2], in_=c13[:, :, 0:2])
                nc.gpsimd.tensor_tensor(
                    out=c23[:, :, 2:8], in0=c13[:, :, 2:8],
                    in1=c13[:, :, 0:6], op=Alu.add,
                )
                c3 = sw_pool.tile([128, TPS * NCAND], f32, tag="c3")
                c33 = seg(c3)
                nc.gpsimd.tensor_copy(out=c33[:, :, 0:4], in_=c23[:, :, 0:4])
                nc.gpsimd.tensor_tensor(
                    out=c33[:, :, 4:8], in0=c23[:, :, 4:8],
                    in1=c23[:, :, 0:4], op=Alu.add,
                )
                # tau = max_j (csum_j - 1)/(j+1) = max_j csum_j*kj - kj
                mm = sw_pool.tile([128, TPS * NCAND], f32, tag="mm")
                nc.gpsimd.tensor_tensor(
                    out=mm, in0=c3, in1=kj_sb[:, :], op=Alu.mult
                )
                mm2 = sw_pool.tile([128, TPS * NCAND], f32, tag="mm2")
                nc.gpsimd.tensor_tensor(
                    out=mm2, in0=mm, in1=kj_sb[:, :], op=Alu.subtract
                )
                # free-axis reduce is Vector-only (GpSimd reduces partitions)
                nc.vector.tensor_reduce(
                    ntau_sb[:, s * TPS:(s + 1) * TPS],
                    seg(mm2),
                    axis=mybir.AxisListType.X, op=Alu.max, negate=True,
                )

                # software pipeline: the PREVIOUS slab's finals are emitted
                # here so they overlap this slab's PE/DVE work.
                if prev is not None:
                    ps, pz = prev
                    o = o_pool.tile([128, TPS * F], bf16)
                    for j in range(TPS):
                        t = ps * TPS + j
                        nc.scalar.activation(
                            o[:, j * F:(j + 1) * F], pz[j], Act.Relu,
                            bias=ntau_sb[:, t:t + 1], scale=1.0,
                        )
                    dst = out_d[
                        ps * TPS * 128:(ps + 1) * TPS * 128, :
                    ].rearrange("(j p) f -> p j f", j=TPS)
                    nc.sync.dma_start(
                        out=dst, in_=o.rearrange("p (j f) -> p j f", j=TPS)
                    )
                prev = (s, z_list)

            # epilogue: finals for the last slab
            ps, pz = prev
            o = o_pool.tile([128, TPS * F], bf16)
            for j in range(TPS):
                t = ps * TPS + j
                nc.scalar.activation(
                    o[:, j * F:(j + 1) * F], pz[j], Act.Relu,
                    bias=ntau_sb[:, t:t + 1], scale=1.0,
                )
            dst = out_d[
                ps * TPS * 128:(ps + 1) * TPS * 128, :
            ].rearrange("(j p) f -> p j f", j=TPS)
            nc.sync.dma_start(
                out=dst, in_=o.rearrange("p (j f) -> p j f", j=TPS)
            )
    nc.finalize()
    return nc


def _sparsemax_rows(v):
    """Exact row-wise sparsemax of v [R, F] (float64)."""
    vs = -np.sort(-v, axis=-1)
    cs = np.cumsum(vs, axis=-1)
    kk = np.arange(1, v.shape[-1] + 1)
    ks = ((1.0 + kk * vs) > cs).sum(-1)
    tau = (np.take_along_axis(cs, (ks - 1)[:, None], -1) - 1.0) / ks[:, None]
    return np.maximum(v - tau, 0.0)


def kernel(**inputs):
    import ml_dtypes

    bf = ml_dtypes.bfloat16
    x = np.asarray(inputs["inputs"], dtype=np.float32)
    W = np.asarray(inputs["W"], dtype=np.float64)
    b = np.asarray(inputs["b"], dtype=np.float64)
    gamma = np.asarray(inputs["gamma"], dtype=np.float64)
    beta = np.asarray(inputs["beta"], dtype=np.float64)
    mmean = np.asarray(inputs["moving_mean"], dtype=np.float64)
    mvar = np.asarray(inputs["moving_var"], dtype=np.float64)

    # fold BatchNorm (inference) into the dense layer
    s = gamma / np.sqrt(mvar + BN_EPS)
    w_fold = (W * s[None, :]).astype(np.float32)
    cvec = ((b - mmean) * s + beta).astype(np.float32)

    w_bf = w_fold.astype(bf)
    xt = np.ascontiguousarray(x.T).astype(bf)     # [D, B] bf16
    kj = np.tile(
        (1.0 / np.arange(1, NCAND + 1)).astype(np.float32), TPS
    )[None, :]

    in_maps = [
        {
            "xt": np.ascontiguousarray(xt[:, c * RPC:(c + 1) * RPC]),
            "w": w_bf,
            "kj": kj,
        }
        for c in range(NCORES)
    ]

    use_bias = bool(np.any(cvec != 0.0))
    if use_bias:
        c_hi = cvec.astype(bf)
        c_lo = (cvec - c_hi.astype(np.float32)).astype(bf)
        cv2 = np.stack([c_hi, c_lo], axis=0)      # [2, F] bf16
        for c in range(NCORES):
            in_maps[c]["cv"] = cv2
    prior = np.asarray(inputs["prior"], dtype=np.float32)
    use_prior = bool(np.any(prior != 1.0))
    if use_prior:
        for c in range(NCORES):
            in_maps[c]["prior"] = np.ascontiguousarray(
                prior[c * RPC:(c + 1) * RPC]
            )

    key = ("nc", use_bias, use_prior)
    if key not in _CACHE:
        _CACHE[key] = _build_program(use_bias, use_prior)

    # If BASS_TRACE is set but the NTFF glue module is absent in this
    # environment, bass_utils would crash on import; stub it so tracing is
    # skipped gracefully and the run proceeds.
    try:
        import antenv.axon_hooks  # noqa: F401
    except ImportError:
        import sys as _sys
        import types as _types

        try:
            import antenv as _antenv

            _stub = _types.ModuleType("antenv.axon_hooks")
            _stub.get_axon_ntff_profile_hook = lambda: None
            _stub.set_axon_ntff_profile_hook = lambda h: None
            _sys.modules["antenv.axon_hooks"] = _stub
            _antenv.axon_hooks = _stub
        except ImportError:
            pass

    from concourse.bass_utils import run_bass_kernel_spmd

    res = run_bass_kernel_spmd(_CACHE[key], in_maps, core_ids=list(range(NCORES)))
    _CACHE["last_results"] = res
    mask = np.concatenate(
        [res.results[c]["out"] for c in range(NCORES)], axis=0
    ).astype(np.float32)

    # Host fix: rows whose mask sums above 1 had tau_hat < tau* (support
    # larger than NCAND); exact sparsemax on the written row recovers the
    # true projection. Over-flagging is a no-op.
    rowsum = mask.sum(axis=1)
    rows = np.where(rowsum > 1.0 + SUM_TOL)[0]
    if rows.size:
        mask[rows] = _sparsemax_rows(mask[rows].astype(np.float64)).astype(
            np.float32
        )
    return mask


# revision 9
# speedup vs baseline: 1.1059x; 1.1059x over previous
"""Trainium2 Bass kernel for the AttentiveTransformer block:
    mask = sparsemax(BN(inputs @ W + b) * prior)

Contract: kernel(**inputs) takes FULL unsharded numpy inputs and returns the
FULL [65536, 512] float32 output. The batch axis is sharded over 8
NeuronCores (pure data parallelism, 8192 rows each); the small Dense/BN
params are replicated to every core. No cross-core communication is needed
(sparsemax is row-wise).

Host-side prep (cheap, O(B*D)): BatchNorm (inference) is folded into the
dense layer; inputs are pre-transposed to [D, B] bf16 so the contraction dim
lands on partitions with no on-device transpose. A single bf16 matmul is
used (z error ~5e-3 absmax, inside the 2e-2 budget).

Device algorithm per 128-row tile (rows on partitions, F=512 on free axis):
  1. PE: one bf16 matmul -> PSUM fp32 (z stays resident in PSUM).
  2. DVE: one max8 over the 512 columns -> top-8 candidates in SBUF.
  3. GpSimd (Pool): batched simplex-threshold math on the candidates,
     tau_hat = max_j (csum_j - 1)/(j+1) over j<ncand. Because tau from ANY
     value-subset of a row lower-bounds the true sparsemax tau, tau_hat <=
     tau* always; rows with support > ncand get tau_hat < tau*.
  4. ACT: out = Relu(z + (-tau_hat)) straight from PSUM, written bf16.
  5. Batched 4-tile slab DMAs in/out on the sync HWDGE ring.

Host post-pass: rows whose written mask sums to > 1 + eps are exactly those
where tau_hat < tau* (plus harmless rounding flags); for those (~7%) the
host runs an exact sparsemax on the row's nonzeros, which provably recovers
the true projection of the device z. Re-fixing an already-correct row is a
no-op, so over-flagging is safe. End-to-end absmax err ~7e-3 (gate 2e-2).

Input-dependent specialization (checked on host at call time):
  * folded BN bias is zero for this problem -> bias rank-1 matmul elided;
  * `prior` is all-ones (spec fill=ones) -> prior load/multiply skipped.
  Program variants exist for both non-default cases.
"""

import numpy as np

B, D, F = 65536, 128, 512
NCORES = 8
RPC = B // NCORES        # rows per core
NT = RPC // 128          # 128-row tiles per core (64)
TPS = 4                  # tiles per slab (= PSUM batch for threshold math)
NS = NT // TPS           # slabs per core (16)
NCAND = 8                # top-k candidates per row for tau_hat
SEG = 12                 # candidate segment stride: 4 zero guards + 8 values
G = 4                    # guard columns per segment
BN_EPS = 1e-3
SUM_TOL = 2e-3           # host-fix flag threshold on row sums

_CACHE = {}


def _build_program(use_bias, use_prior):
    import concourse.bass as bass
    import concourse.bacc as bacc
    import concourse.mybir as mybir
    from concourse.tile import TileContext

    f32 = mybir.dt.float32
    bf16 = mybir.dt.bfloat16
    Alu = mybir.AluOpType
    Act = mybir.ActivationFunctionType

    nc = bacc.Bacc("TRN2", target_bir_lowering=False)
    xt_d = nc.dram_tensor("xt", [D, RPC], bf16, kind="ExternalInput")
    w_d = nc.dram_tensor("w", [D, F], bf16, kind="ExternalInput")
    kj_d = nc.dram_tensor("kj", [1, TPS * SEG], f32, kind="ExternalInput")
    if use_bias:
        cv_d = nc.dram_tensor("cv", [2, F], bf16, kind="ExternalInput")
    if use_prior:
        pr_d = nc.dram_tensor("prior", [RPC, F], f32, kind="ExternalInput")
    out_d = nc.dram_tensor("out", [RPC, F], bf16, kind="ExternalOutput")

    with TileContext(nc) as tc:
        with (
            tc.tile_pool(name="consts", bufs=1) as consts,
            tc.tile_pool(name="xin", bufs=3) as xin_pool,
            tc.tile_pool(name="psum", bufs=8, space="PSUM") as psum_pool,
            tc.tile_pool(name="obuf", bufs=3) as o_pool,
            tc.tile_pool(name="smallw", bufs=2) as sw_pool,
            tc.tile_pool(name="smallt", bufs=2) as st_pool,
            tc.tile_pool(name="zbuf", bufs=2 * TPS + 2) as z_pool,
        ):
            w_sb = consts.tile([D, F], bf16)
            nc.scalar.dma_start(out=w_sb, in_=w_d[:, :])
            if use_bias:
                cv_sb = consts.tile([2, F], bf16)
                nc.sync.dma_start(out=cv_sb, in_=cv_d[:, :])
                ones_sb = consts.tile([2, D], bf16)
                nc.vector.memset(ones_sb, 1.0)
            # 1/(j+1) rule coefficients in the guarded 12-wide layout,
            # replicated to all 128 partitions (scalar HWDGE ring)
            kj_sb = consts.tile([128, TPS * SEG], f32)
            kj_bcast = bass.AP(
                tensor=kj_d, offset=0, ap=[[0, 128]] + kj_d[0:1, :].ap[1:]
            )
            nc.scalar.dma_start(out=kj_sb, in_=kj_bcast)
            # Candidates live in 12-wide segments: 4 zero guard columns then
            # the 8 max8 values. The guards make the Hillis-Steele cumsum
            # ladder 3 plain shifted adds (reads of the guard zeros replace
            # the prefix copies). Guards are memset once; ladder writes only
            # cols G..SEG-1, so they stay zero. Issuing the memsets first
            # also absorbs the gpsimd first-op warmup during const loads.
            cand_ab = [
                consts.tile([128, TPS * SEG], f32, name=f"cand{i}")
                for i in range(2)
            ]
            c1_sb = consts.tile([128, TPS * SEG], f32)
            c2_sb = consts.tile([128, TPS * SEG], f32)
            mm_sb = consts.tile([128, TPS * SEG], f32)
            mm2_sb = consts.tile([128, TPS * SEG], f32)
            nc.gpsimd.memset(cand_ab[0], 0.0)
            nc.gpsimd.memset(cand_ab[1], 0.0)
            nc.gpsimd.memset(c1_sb, 0.0)
            nc.gpsimd.memset(c2_sb, 0.0)
            ntau_sb = consts.tile([128, NT], f32)

            prev = None
            for s in range(NS):
                cand_sb = cand_ab[s % 2]  # ping-pong vs Pool reads (WAR)
                xin = xin_pool.tile([D, TPS * 128], bf16)
                nc.sync.dma_start(
                    out=xin, in_=xt_d[:, s * TPS * 128:(s + 1) * TPS * 128]
                )
                z_list = []
                for j in range(TPS):
                    t = s * TPS + j
                    xps = psum_pool.tile([128, F], f32)
                    nc.tensor.matmul(
                        xps, lhsT=xin[:, j * 128:(j + 1) * 128], rhs=w_sb[:, :],
                        start=True, stop=not use_bias,
                    )
                    if use_bias:
                        nc.tensor.matmul(
                            xps, lhsT=ones_sb[:, :], rhs=cv_sb[:, :],
                            start=False, stop=True,
                        )
                    if use_prior:
                        pr_t = xin_pool.tile([128, F], f32, tag="pr")
                        nc.sync.dma_start(
                            out=pr_t, in_=pr_d[t * 128:(t + 1) * 128, :]
                        )
                        z = z_pool.tile([128, F], f32)
                        nc.vector.tensor_tensor(
                            out=z, in0=xps, in1=pr_t, op=Alu.mult
                        )
                    else:
                        z = xps
                    nc.vector.max(
                        out=cand_sb[:, j * SEG + G:(j + 1) * SEG], in_=z
                    )
                    z_list.append(z)

                # ---- batched threshold math on GpSimd (SBUF-only engine) ----
                # Per-segment cumsum: Hillis-Steele ladder of 3 shifted adds
                # reading the zero guards in place of prefix copies.
                seg = lambda ap: ap.rearrange("p (t s) -> p t s", s=SEG)
                cnd3, c13, c23 = seg(cand_sb), seg(c1_sb), seg(c2_sb)
                nc.gpsimd.tensor_tensor(
                    out=c13[:, :, G:SEG], in0=cnd3[:, :, G:SEG],
                    in1=cnd3[:, :, G - 1:SEG - 1], op=Alu.add,
                )
                nc.gpsimd.tensor_tensor(
                    out=c23[:, :, G:SEG], in0=c13[:, :, G:SEG],
                    in1=c13[:, :, G - 2:SEG - 2], op=Alu.add,
                )
                c33 = seg(cand_sb)  # reuse cand as the ladder's last rung
                nc.gpsimd.tensor_tensor(
                    out=c33[:, :, G:SEG], in0=c23[:, :, G:SEG],
                    in1=c23[:, :, G - 4:SEG - 4], op=Alu.add,
                )
                # tau = max_j (csum_j - 1)/(j+1) = max_j csum_j*kj - kj
                nc.gpsimd.tensor_tensor(
                    out=mm_sb, in0=cand_sb, in1=kj_sb[:, :], op=Alu.mult
                )
                nc.gpsimd.tensor_tensor(
                    out=mm2_sb, in0=mm_sb, in1=kj_sb[:, :], op=Alu.subtract
                )
                # free-axis reduce is Vector-only (GpSimd reduces partitions)
                nc.vector.tensor_reduce(
                    ntau_sb[:, s * TPS:(s + 1) * TPS],
                    seg(mm2_sb)[:, :, G:SEG],
                    axis=mybir.AxisListType.X, op=Alu.max, negate=True,
                )

                # software pipeline: the PREVIOUS slab's finals are emitted
                # here so they overlap this slab's PE/DVE work.
                if prev is not None:
                    ps, pz = prev
                    o = o_pool.tile([128, TPS * F], bf16)
                    for j in range(TPS):
                        t = ps * TPS + j
                        nc.scalar.activation(
                            o[:, j * F:(j + 1) * F], pz[j], Act.Relu,
                            bias=ntau_sb[:, t:t + 1], scale=1.0,
                        )
                    dst = out_d[
                        ps * TPS * 128:(ps + 1) * TPS * 128, :
                    ].rearrange("(j p) f -> p j f", j=TPS)
                    nc.sync.dma_start(
                        out=dst, in_=o.rearrange("p (j f) -> p j f", j=TPS)
                    )
                prev = (s, z_list)

            # epilogue: finals for the last slab
            ps, pz = prev
            o = o_pool.tile([128, TPS * F], bf16)
            for j in range(TPS):
                t = ps * TPS + j
                nc.scalar.activation(
                    o[:, j * F:(j + 1) * F], pz[j], Act.Relu,
                    bias=ntau_sb[:, t:t + 1], scale=1.0,
                )
            dst = out_d[
                ps * TPS * 128:(ps + 1) * TPS * 128, :
            ].rearrange("(j p) f -> p j f", j=TPS)
            nc.sync.dma_start(
                out=dst, in_=o.rearrange("p (j f) -> p j f", j=TPS)
            )
    nc.finalize()
    return nc


def _sparsemax_rows(v):
    """Exact row-wise sparsemax of v [R, F] (float64)."""
    vs = -np.sort(-v, axis=-1)
    cs = np.cumsum(vs, axis=-1)
    kk = np.arange(1, v.shape[-1] + 1)
    ks = ((1.0 + kk * vs) > cs).sum(-1)
    tau = (np.take_along_axis(cs, (ks - 1)[:, None], -1) - 1.0) / ks[:, None]
    return np.maximum(v - tau, 0.0)


def kernel(**inputs):
    import ml_dtypes

    bf = ml_dtypes.bfloat16
    x = np.asarray(inputs["inputs"], dtype=np.float32)
    W = np.asarray(inputs["W"], dtype=np.float64)
    b = np.asarray(inputs["b"], dtype=np.float64)
    gamma = np.asarray(inputs["gamma"], dtype=np.float64)
    beta = np.asarray(inputs["beta"], dtype=np.float64)
    mmean = np.asarray(inputs["moving_mean"], dtype=np.float64)
    mvar = np.asarray(inputs["moving_var"], dtype=np.float64)

    # fold BatchNorm (inference) into the dense layer
    s = gamma / np.sqrt(mvar + BN_EPS)
    w_fold = (W * s[None, :]).astype(np.float32)
    cvec = ((b - mmean) * s + beta).astype(np.float32)

    w_bf = w_fold.astype(bf)
    xt = np.ascontiguousarray(x.T).astype(bf)     # [D, B] bf16
    kj_seg = np.zeros(SEG, dtype=np.float32)
    kj_seg[G:] = 1.0 / np.arange(1, NCAND + 1)
    kj = np.tile(kj_seg, TPS)[None, :]

    in_maps = [
        {
            "xt": np.ascontiguousarray(xt[:, c * RPC:(c + 1) * RPC]),
            "w": w_bf,
            "kj": kj,
        }
        for c in range(NCORES)
    ]

    use_bias = bool(np.any(cvec != 0.0))
    if use_bias:
        c_hi = cvec.astype(bf)
        c_lo = (cvec - c_hi.astype(np.float32)).astype(bf)
        cv2 = np.stack([c_hi, c_lo], axis=0)      # [2, F] bf16
        for c in range(NCORES):
            in_maps[c]["cv"] = cv2
    prior = np.asarray(inputs["prior"], dtype=np.float32)
    use_prior = bool(np.any(prior != 1.0))
    if use_prior:
        for c in range(NCORES):
            in_maps[c]["prior"] = np.ascontiguousarray(
                prior[c * RPC:(c + 1) * RPC]
            )

    key = ("nc", use_bias, use_prior)
    if key not in _CACHE:
        _CACHE[key] = _build_program(use_bias, use_prior)

    # If BASS_TRACE is set but the NTFF glue module is absent in this
    # environment, bass_utils would crash on import; stub it so tracing is
    # skipped gracefully and the run proceeds.
    try:
        import antenv.axon_hooks  # noqa: F401
    except ImportError:
        import sys as _sys
        import types as _types

        try:
            import antenv as _antenv

            _stub = _types.ModuleType("antenv.axon_hooks")
            _stub.get_axon_ntff_profile_hook = lambda: None
            _stub.set_axon_ntff_profile_hook = lambda h: None
            _sys.modules["antenv.axon_hooks"] = _stub
            _antenv.axon_hooks = _stub
        except ImportError:
            pass

    from concourse.bass_utils import run_bass_kernel_spmd

    res = run_bass_kernel_spmd(_CACHE[key], in_maps, core_ids=list(range(NCORES)))
    _CACHE["last_results"] = res
    mask = np.concatenate(
        [res.results[c]["out"] for c in range(NCORES)], axis=0
    ).astype(np.float32)

    # Host fix: rows whose mask sums above 1 had tau_hat < tau* (support
    # larger than NCAND); exact sparsemax on the written row recovers the
    # true projection. Over-flagging is a no-op.
    rowsum = mask.sum(axis=1)
    rows = np.where(rowsum > 1.0 + SUM_TOL)[0]
    if rows.size:
        mask[rows] = _sparsemax_rows(mask[rows].astype(np.float64)).astype(
            np.float32
        )
    return mask
